# revision 37
# baseline (speedup 1.0000x reference)
"""Trainium2 Bass kernel for nn_CausalSelfPoM (B=4, T=4096, C=1024, H=2048, DEGREE=2).

Sharding: 2 data-parallel groups over batch pairs x 4-way tensor-parallel over
the hidden dim H. Core k handles batches {2g, 2g+1} (g = k//4) and H channels
[q*512, (q+1)*512) (q = k%4).

Everything on-chip lives in (H x T) orientation (hidden channels on SBUF
partitions, tokens along the free dim):
  - s / c0 / c1 come from fp32r (TF32) matmuls with W chunks stationary and
    x^T chunks moving,
  - poly = c0 * (1 + c1),
  - the causal cumsum is a DVE tensor_tensor_scan along the free (token) dim,
    carry-chained across token chunks via initial=prev[:, -1:],
  - m = silu(s) * (1/t) * cumsum is written as fp32r and is directly the
    stationary operand of the output projection,
  - partial outputs (contraction over the local H shard) are pair...
    4-way ReduceScattered across the TP group and DMA'd to the external output.
"""

import os
import numpy as np

import concourse.bass as bass
import concourse.tile as tile
from concourse import bacc, mybir
from concourse.bass_utils import run_bass_kernel_spmd

F32 = mybir.dt.float32
F32R = mybir.dt.float32r
BF16 = mybir.dt.bfloat16
F16 = mybir.dt.float16
PDT = F16  # dtype of the partial-output reduce path (fp16: values are O(1))
AF = mybir.ActivationFunctionType
ALU = mybir.AluOpType

B, T, C = 4, 4096, 1024
H = 2048
DH = 4096  # DEGREE * H
N_CORES = 8
TPW = 4                  # tensor-parallel width (cores per replica group)
NDP = N_CORES // TPW     # data-parallel groups
BPG = B // NDP           # batches per group (2)
HSH = H // TPW           # H shard per core (512)
NHT = HSH // 128         # h-tiles per core (4)
KC = C // 128            # contraction chunks (8)
TCH = 512                # tokens per chunk
TOK = BPG * T            # tokens per core (8192)
NCH = TOK // TCH         # chunks per core (16)
CPB = T // TCH           # chunks per batch (8)
CC_SCHED = [2, 2, 2, 2, 2, 2, 3, 1]  # chunks per collective call
NCC = len(CC_SCHED)
CC_START = [sum(CC_SCHED[:i]) for i in range(NCC)]  # first chunk of each call
CO = C                   # output channels


def _tf32_round(x: np.ndarray) -> np.ndarray:
    u = np.ascontiguousarray(x, dtype=np.float32).view(np.uint32)
    r = u + (0x0FFF + ((u >> 13) & np.uint32(1)))
    r &= np.uint32(0xFFFFE000)
    return r.view(np.float32)


def _build():
    nc = bacc.Bacc(None, target_bir_lowering=False, num_devices=N_CORES)

    # per-chunk-contiguous layout: each (128, KC*TCH) chunk tile reads 16KB
    # of contiguous dram per partition (large DMA descriptors)
    xt = nc.declare_dram_parameter("xt", [NCH, 128, KC * TCH], F16, isOutput=False)
    w_sel = nc.declare_dram_parameter("w_sel", [128, KC, HSH], F16, isOutput=False)
    w_c0 = nc.declare_dram_parameter("w_c0", [128, KC, HSH], F16, isOutput=False)
    w_c1 = nc.declare_dram_parameter("w_c1", [128, KC, HSH], F16, isOutput=False)
    w_out = nc.declare_dram_parameter("w_out", [128, NHT, CO], F16, isOutput=False)
    r_bc = nc.declare_dram_parameter("r_bc", [128, T], F32, isOutput=False)

    out_ext = nc.declare_dram_parameter("out", [TOK // TPW, CO], PDT, isOutput=True)

    groups = [list(range(g * TPW, (g + 1) * TPW)) for g in range(NDP)]

    with tile.TileContext(nc) as tc:
        with (
            tc.tile_pool(name="consts", bufs=1) as consts,
            tc.tile_pool(name="sb", bufs=2) as sb,
            tc.tile_pool(name="ps", bufs=2, space="PSUM") as ps,
            tc.tile_pool(name="dram", bufs=1, space="DRAM") as dram,
        ):
            # load in consumption order, split by kc halves so the first
            # matmuls only wait for the first megabyte; w_out/r on the qACT
            # queue so the first x chunks aren't delayed on qSP
            # interleave the first chunk-0 x slices with the w_c1 halves (the
            # first psum group needs exactly these 4MB), then the rest in
            # consumption order
            kh_ = KC // 2
            w_c1_sb = consts.tile([128, KC, HSH], F16)
            x_first = sb.tile([128, KC, TCH], F16, tag="x", bufs=3, name="x_first")
            xj0 = xt[0].rearrange("p (kc t) -> p kc t", kc=KC)
            nc.sync.dma_start(out=w_c1_sb[:, :kh_], in_=w_c1[:, :kh_, :])
            nc.sync.dma_start(out=x_first[:, :kh_, :], in_=xj0[:, :kh_, :])
            nc.sync.dma_start(out=w_c1_sb[:, kh_:], in_=w_c1[:, kh_:, :])
            nc.sync.dma_start(out=x_first[:, kh_:, :], in_=xj0[:, kh_:, :])
            w_c0_sb = consts.tile([128, KC, HSH], F16)
            nc.sync.dma_start(out=w_c0_sb[:, :kh_], in_=w_c0[:, :kh_, :])
            w_sel_sb = consts.tile([128, KC, HSH], F16)
            nc.sync.dma_start(out=w_sel_sb[:, :kh_], in_=w_sel[:, :kh_, :])
            nc.sync.dma_start(out=w_c0_sb[:, kh_:], in_=w_c0[:, kh_:, :])
            nc.sync.dma_start(out=w_sel_sb[:, kh_:], in_=w_sel[:, kh_:, :])
            w_out_sb = consts.tile([128, NHT, CO], F16)
            nc.scalar.dma_start(out=w_out_sb, in_=w_out[:, :, :])
            r_sb = consts.tile([128, T], F32)
            nc.scalar.dma_start(out=r_sb, in_=r_bc[:, :])

            # one partial/red tile pair per collective call: slices of a single
            # big tile would create false write-after-read deps against the
            # in-flight ReduceScatter and stall the whole pipeline
            partials = []
            reds = []
            for c in range(NCC):
                partial_c = dram.tile([CC_SCHED[c] * TCH, CO], PDT, name=f"partial{c}")
                red_c = dram.tile([CC_SCHED[c] * TCH // TPW, CO], PDT, name=f"red{c}")
                partials.append(partial_c)
                reds.append(red_c)

            agg_prev = [None] * NHT

            for j in range(NCH):
                if j == 0:
                    x_sb = x_first
                else:
                    x_sb = sb.tile([128, KC, TCH], F16, tag="x", bufs=3)
                    # split by kc halves: the first half's matmuls can start
                    # while the second half is still in flight
                    xj = xt[j].rearrange("p (kc t) -> p kc t", kc=KC)
                    nc.sync.dma_start(
                        out=x_sb[:, : KC // 2, :], in_=xj[:, : KC // 2, :]
                    )
                    nc.sync.dma_start(
                        out=x_sb[:, KC // 2 :, :], in_=xj[:, KC // 2 :, :]
                    )

                m_all = sb.tile([128, NHT, TCH], F16, tag="m", bufs=2)

                tr = j % CPB  # chunk index within the batch (for r slice)

                for i in range(NHT):
                    hsl = slice(i * 128, (i + 1) * 128)

                    c1_ps = ps.tile([128, TCH], F32, tag="c1_ps")
                    for kc in range(KC):
                        nc.tensor.matmul(
                            c1_ps,
                            w_c1_sb[:, kc, hsl],
                            x_sb[:, kc, :],
                            start=(kc == 0),
                            stop=(kc == KC - 1),
                        )
                    c0_ps = ps.tile([128, TCH], F32, tag="c0_ps")
                    for kc in range(KC):
                        nc.tensor.matmul(
                            c0_ps,
                            w_c0_sb[:, kc, hsl],
                            x_sb[:, kc, :],
                            start=(kc == 0),
                            stop=(kc == KC - 1),
                        )
                    s_ps = ps.tile([128, TCH], F32, tag="s_ps")
                    for kc in range(KC):
                        nc.tensor.matmul(
                            s_ps,
                            w_sel_sb[:, kc, hsl],
                            x_sb[:, kc, :],
                            start=(kc == 0),
                            stop=(kc == KC - 1),
                        )

                    # poly = c0 * (1 + c1)
                    c1p_sb = sb.tile([128, TCH], F32, tag="c1p")
                    nc.scalar.activation(c1p_sb, c1_ps, AF.Copy, bias=1.0)
                    poly_sb = sb.tile([128, TCH], F32, tag="poly")
                    nc.vector.tensor_tensor(poly_sb, c0_ps, c1p_sb, ALU.mult)

                    # causal cumsum along tokens, chained across chunks
                    agg_sb = sb.tile([128, TCH], F32, tag=f"agg{i}")
                    init = 0.0 if tr == 0 else agg_prev[i][:, TCH - 1 : TCH]
                    nc.vector.tensor_tensor_scan(
                        agg_sb,
                        poly_sb,
                        poly_sb,
                        init,
                        op0=ALU.add,
                        op1=ALU.bypass,
                    )
                    agg_prev[i] = agg_sb

                    # m = silu(s) * r * agg   (r = 1/(t+1))
                    s_sb = sb.tile([128, TCH], F32, tag="s_sb")
                    nc.scalar.activation(s_sb, s_ps, AF.Silu)
                    sr_sb = sb.tile([128, TCH], F32, tag="sr")
                    nc.vector.tensor_tensor(
                        sr_sb, s_sb, r_sb[:, tr * TCH : (tr + 1) * TCH], ALU.mult
                    )
                    nc.vector.tensor_tensor(m_all[:, i, :], sr_sb, agg_sb, ALU.mult)

                # output projection for this chunk: out[t, c] = sum_h m[h,t] Wout[h,c]
                c = max(i_ for i_ in range(NCC) if CC_START[i_] <= j)
                jr = j - CC_START[c]  # chunk index within the collective group
                for tb in range(TCH // 128):
                    tsl = slice(tb * 128, (tb + 1) * 128)
                    outp = [None] * (CO // 512)
                    for co in range(CO // 512):
                        outp[co] = ps.tile(
                            [128, 512], F32, tag=f"out_ps{co}", bufs=1, name=f"outp{co}"
                        )
                    # i outer / co inner: two matmuls share each stationary
                    # m tile, halving the fp32r weight-load overhead
                    for i in range(NHT):
                        for co in range(CO // 512):
                            nc.tensor.matmul(
                                outp[co],
                                m_all[:, i, tsl],
                                w_out_sb[:, i, co * 512 : (co + 1) * 512],
                                start=(i == 0),
                                stop=(i == NHT - 1),
                            )
                    # combine both co halves into one (128, 1024) tile so the
                    # partial write lands as full 4KB dram rows per partition
                    out_sb = sb.tile([128, CO], PDT, tag="out_sb", bufs=3)
                    for co in range(CO // 512):
                        nc.scalar.activation(
                            out_sb[:, co * 512 : (co + 1) * 512], outp[co], AF.Copy
                        )
                    # qACT HW queue: keep partial writes off the qSP queue
                    # so x-tile prefetches are never stuck behind them
                    nc.scalar.dma_start(
                        out=partials[c][
                            jr * TCH + tb * 128 : jr * TCH + (tb + 1) * 128, :
                        ],
                        in_=out_sb,
                    )

                # reduce-scatter a completed group of chunks across the TP group
                if j + 1 == CC_START[c] + CC_SCHED[c]:
                    nc.gpsimd.collective_compute(
                        "ReduceScatter",
                        ALU.add,
                        replica_groups=groups,
                        ins=[partials[c].opt()],
                        outs=[reds[c].opt()],
                    )
                    # gpsimd queue: a sync-queue DMA here would wait on the
                    # collective and head-of-line block later x-tile loads
                    r0 = CC_START[c] * TCH // TPW
                    nc.gpsimd.dma_start(
                        out=out_ext[r0 : r0 + CC_SCHED[c] * TCH // TPW, :],
                        in_=reds[c][:, :],
                    )

    nc.compile()
    return nc


_NC_CACHE = None


def _in_maps(x, W_sel, W_agg, W_out, sqk_q, sqk_k):
    x = np.asarray(x, dtype=np.float32)
    W_sel = np.asarray(W_sel, dtype=np.float32)
    W_agg = np.asarray(W_agg, dtype=np.float32)
    W_out = np.asarray(W_out, dtype=np.float32)
    sqk_q = np.asarray(sqk_q, dtype=np.float32)
    sqk_k = np.asarray(sqk_k, dtype=np.float32)

    # fold the per-channel scales into the projection weights
    W_sel_e = (W_sel * sqk_q[None, :]).astype(np.float16)
    W_agg_e = (W_agg * sqk_k[None, :]).astype(np.float16)
    W_out_r = W_out.astype(np.float16)
    x_r = x.astype(np.float16)

    r_bc = np.broadcast_to(
        (1.0 / np.arange(1, T + 1, dtype=np.float32))[None, :], (128, T)
    ).copy()

    def fold_k(a, ncol):
        # (C, ncol) -> (128, KC, ncol) with C = KC*128 split across partitions
        return np.ascontiguousarray(a.reshape(KC, 128, ncol).transpose(1, 0, 2))

    in_maps = []
    xt_cache = {}
    for k in range(N_CORES):
        g, q = k // TPW, k % TPW
        if g not in xt_cache:
            xg = np.concatenate([x_r[2 * g].T, x_r[2 * g + 1].T], axis=1)  # (C, TOK)
            # (NCH, 128, KC*TCH): per chunk, per partition, contiguous 16KB
            xt_cache[g] = np.ascontiguousarray(
                xg.reshape(KC, 128, NCH, TCH)
                .transpose(2, 1, 0, 3)
                .reshape(NCH, 128, KC * TCH)
            )
        in_maps.append(
            {
                "xt": xt_cache[g],
                "w_sel": fold_k(W_sel_e[:, q * HSH : (q + 1) * HSH], HSH),
                "w_c0": fold_k(W_agg_e[:, q * HSH : (q + 1) * HSH], HSH),
                "w_c1": fold_k(W_agg_e[:, H + q * HSH : H + (q + 1) * HSH], HSH),
                "w_out": np.ascontiguousarray(
                    W_out_r[q * HSH : (q + 1) * HSH, :]
                    .reshape(NHT, 128, CO)
                    .transpose(1, 0, 2)
                ),
                "r_bc": r_bc,
            }
        )
    return in_maps


def _assemble(res):
    # call c covers local tokens [CC_START[c]*TCH, ...); within the call the
    # ReduceScatter hands rank q the q-th quarter of those rows
    out = np.empty((B, T, C), dtype=np.float32)
    for g in range(NDP):
        for c in range(NCC):
            rows = CC_SCHED[c] * TCH // TPW
            r0 = CC_START[c] * TCH // TPW
            for q in range(TPW):
                piece = np.asarray(
                    res.results[TPW * g + q]["out"][r0 : r0 + rows], dtype=np.float32
                )
                lstart = CC_START[c] * TCH + q * rows
                b = 2 * g + lstart // T
                t0 = lstart % T
                out[b, t0 : t0 + rows] = piece
    return out


def kernel(x, W_sel, W_agg, W_out, sqk_q, sqk_k):
    global _NC_CACHE
    in_maps = _in_maps(x, W_sel, W_agg, W_out, sqk_q, sqk_k)
    if _NC_CACHE is None:
        _NC_CACHE = _build()
    res = run_bass_kernel_spmd(_NC_CACHE, in_maps, core_ids=list(range(N_CORES)))
    return _assemble(res)


def run_traced(x, W_sel, W_agg, W_out, sqk_q, sqk_k):
    """Timed run with NTFF trace; returns BassKernelResults (for test.py)."""
    global _NC_CACHE
    in_maps = _in_maps(x, W_sel, W_agg, W_out, sqk_q, sqk_k)
    if _NC_CACHE is None:
        _NC_CACHE = _build()
    return run_bass_kernel_spmd(
        _NC_CACHE,
        in_maps,
        core_ids=list(range(N_CORES)),
        trace=True,
        trace_cores=list(range(N_CORES)),
    )


# revision 41
# speedup vs baseline: 1.0296x; 1.0296x over previous
"""Trainium2 Bass kernel for nn_CausalSelfPoM (B=4, T=4096, C=1024, H=2048, DEGREE=2).

Sharding: 2 data-parallel groups over batch pairs x 4-way tensor-parallel over
the hidden dim H. Core k handles batches {2g, 2g+1} (g = k//4) and H channels
[q*512, (q+1)*512) (q = k%4).

Everything on-chip lives in (H x T) orientation (hidden channels on SBUF
partitions, tokens along the free dim):
  - s / c0 / c1 come from fp16 matmuls (same 10-bit mantissa as TF32; fp32
    PSUM accumulation) with W chunks stationary and x^T chunks moving,
  - poly = c0 * (1 + c1),
  - the causal cumsum is a DVE tensor_tensor_scan along the free (token) dim,
    carry-chained across token chunks via initial=prev[:, -1:] (fp32),
  - m = silu(s) * (1/t) * cumsum is written as fp16 and is directly the
    stationary operand of the output projection,
  - fp16 partial outputs (contraction over the local H shard) are 4-way
    ReduceScattered across the TP group and DMA'd to the external output;
    the host reassembles and casts to fp32.
"""

import os
import numpy as np

import concourse.bass as bass
import concourse.tile as tile
from concourse import bacc, mybir
from concourse.bass_utils import run_bass_kernel_spmd

F32 = mybir.dt.float32
F32R = mybir.dt.float32r
BF16 = mybir.dt.bfloat16
F16 = mybir.dt.float16
PDT = F16  # dtype of the partial-output reduce path (fp16: values are O(1))
AF = mybir.ActivationFunctionType
ALU = mybir.AluOpType

B, T, C = 4, 4096, 1024
H = 2048
DH = 4096  # DEGREE * H
N_CORES = 8
TPW = 4                  # tensor-parallel width (cores per replica group)
NDP = N_CORES // TPW     # data-parallel groups
BPG = B // NDP           # batches per group (2)
HSH = H // TPW           # H shard per core (512)
NHT = HSH // 128         # h-tiles per core (4)
KC = C // 128            # contraction chunks (8)
TCH = 512                # tokens per chunk
TOK = BPG * T            # tokens per core (8192)
NCH = TOK // TCH         # chunks per core (16)
CPB = T // TCH           # chunks per batch (8)
CC_SCHED = [2, 2, 2, 2, 2, 2, 1, 1, 1, 1]  # chunks per collective call
NCC = len(CC_SCHED)
CC_START = [sum(CC_SCHED[:i]) for i in range(NCC)]  # first chunk of each call
CO = C                   # output channels


def _tf32_round(x: np.ndarray) -> np.ndarray:
    u = np.ascontiguousarray(x, dtype=np.float32).view(np.uint32)
    r = u + (0x0FFF + ((u >> 13) & np.uint32(1)))
    r &= np.uint32(0xFFFFE000)
    return r.view(np.float32)


def _build():
    nc = bacc.Bacc(None, target_bir_lowering=False, num_devices=N_CORES)

    # per-chunk-contiguous layout: each (128, KC*TCH) chunk tile reads 16KB
    # of contiguous dram per partition (large DMA descriptors)
    xt = nc.declare_dram_parameter("xt", [NCH, 128, KC * TCH], F16, isOutput=False)
    w_sel = nc.declare_dram_parameter("w_sel", [128, KC, HSH], F16, isOutput=False)
    w_c0 = nc.declare_dram_parameter("w_c0", [128, KC, HSH], F16, isOutput=False)
    w_c1 = nc.declare_dram_parameter("w_c1", [128, KC, HSH], F16, isOutput=False)
    w_out = nc.declare_dram_parameter("w_out", [128, NHT, CO], F16, isOutput=False)
    r_bc = nc.declare_dram_parameter("r_bc", [128, T], F32, isOutput=False)

    out_ext = nc.declare_dram_parameter("out", [TOK // TPW, CO], PDT, isOutput=True)

    groups = [list(range(g * TPW, (g + 1) * TPW)) for g in range(NDP)]

    with tile.TileContext(nc) as tc:
        with (
            tc.tile_pool(name="consts", bufs=1) as consts,
            tc.tile_pool(name="sb", bufs=2) as sb,
            tc.tile_pool(name="ps", bufs=2, space="PSUM") as ps,
            tc.tile_pool(name="dram", bufs=1, space="DRAM") as dram,
        ):
            # load in consumption order, split by kc halves so the first
            # matmuls only wait for the first megabyte; w_out/r on the qACT
            # queue so the first x chunks aren't delayed on qSP
            # interleave the first chunk-0 x slices with the w_c1 halves (the
            # first psum group needs exactly these 4MB), then the rest in
            # consumption order
            kh_ = KC // 2
            w_c1_sb = consts.tile([128, KC, HSH], F16)
            x_first = sb.tile([128, KC, TCH], F16, tag="x", bufs=5, name="x_first")
            xj0 = xt[0].rearrange("p (kc t) -> p kc t", kc=KC)
            nc.sync.dma_start(out=w_c1_sb[:, :kh_], in_=w_c1[:, :kh_, :])
            nc.sync.dma_start(out=x_first[:, :kh_, :], in_=xj0[:, :kh_, :])
            nc.sync.dma_start(out=w_c1_sb[:, kh_:], in_=w_c1[:, kh_:, :])
            nc.sync.dma_start(out=x_first[:, kh_:, :], in_=xj0[:, kh_:, :])
            w_c0_sb = consts.tile([128, KC, HSH], F16)
            nc.sync.dma_start(out=w_c0_sb[:, :kh_], in_=w_c0[:, :kh_, :])
            w_sel_sb = consts.tile([128, KC, HSH], F16)
            nc.sync.dma_start(out=w_sel_sb[:, :kh_], in_=w_sel[:, :kh_, :])
            nc.sync.dma_start(out=w_c0_sb[:, kh_:], in_=w_c0[:, kh_:, :])
            nc.sync.dma_start(out=w_sel_sb[:, kh_:], in_=w_sel[:, kh_:, :])
            w_out_sb = consts.tile([128, NHT, CO], F16)
            nc.scalar.dma_start(out=w_out_sb, in_=w_out[:, :, :])
            r_sb = consts.tile([128, T], F32)
            nc.scalar.dma_start(out=r_sb, in_=r_bc[:, :])

            # one partial/red tile pair per collective call: slices of a single
            # big tile would create false write-after-read deps against the
            # in-flight ReduceScatter and stall the whole pipeline
            partials = []
            reds = []
            for c in range(NCC):
                partial_c = dram.tile([CC_SCHED[c] * TCH, CO], PDT, name=f"partial{c}")
                red_c = dram.tile([CC_SCHED[c] * TCH // TPW, CO], PDT, name=f"red{c}")
                partials.append(partial_c)
                reds.append(red_c)

            agg_prev = [None] * NHT

            for j in range(NCH):
                if j == 0:
                    x_sb = x_first
                else:
                    x_sb = sb.tile([128, KC, TCH], F16, tag="x", bufs=5)
                    # split by kc halves: the first half's matmuls can start
                    # while the second half is still in flight
                    xj = xt[j].rearrange("p (kc t) -> p kc t", kc=KC)
                    nc.sync.dma_start(
                        out=x_sb[:, : KC // 2, :], in_=xj[:, : KC // 2, :]
                    )
                    nc.sync.dma_start(
                        out=x_sb[:, KC // 2 :, :], in_=xj[:, KC // 2 :, :]
                    )

                m_all = sb.tile([128, NHT, TCH], F16, tag="m", bufs=3)

                tr = j % CPB  # chunk index within the batch (for r slice)

                for i in range(NHT):
                    hsl = slice(i * 128, (i + 1) * 128)

                    c1_ps = ps.tile([128, TCH], F32, tag="c1_ps")
                    for kc in range(KC):
                        nc.tensor.matmul(
                            c1_ps,
                            w_c1_sb[:, kc, hsl],
                            x_sb[:, kc, :],
                            start=(kc == 0),
                            stop=(kc == KC - 1),
                        )
                    c0_ps = ps.tile([128, TCH], F32, tag="c0_ps")
                    for kc in range(KC):
                        nc.tensor.matmul(
                            c0_ps,
                            w_c0_sb[:, kc, hsl],
                            x_sb[:, kc, :],
                            start=(kc == 0),
                            stop=(kc == KC - 1),
                        )
                    s_ps = ps.tile([128, TCH], F32, tag="s_ps")
                    for kc in range(KC):
                        nc.tensor.matmul(
                            s_ps,
                            w_sel_sb[:, kc, hsl],
                            x_sb[:, kc, :],
                            start=(kc == 0),
                            stop=(kc == KC - 1),
                        )

                    # poly = c0 * (1 + c1)
                    c1p_sb = sb.tile([128, TCH], F32, tag="c1p", bufs=3)
                    nc.scalar.activation(c1p_sb, c1_ps, AF.Copy, bias=1.0)
                    poly_sb = sb.tile([128, TCH], F32, tag="poly", bufs=3)
                    nc.vector.tensor_tensor(poly_sb, c0_ps, c1p_sb, ALU.mult)

                    # causal cumsum along tokens, chained across chunks
                    agg_sb = sb.tile([128, TCH], F32, tag=f"agg{i}")
                    init = 0.0 if tr == 0 else agg_prev[i][:, TCH - 1 : TCH]
                    nc.vector.tensor_tensor_scan(
                        agg_sb,
                        poly_sb,
                        poly_sb,
                        init,
                        op0=ALU.add,
                        op1=ALU.bypass,
                    )
                    agg_prev[i] = agg_sb

                    # m = silu(s) * r * agg   (r = 1/(t+1))
                    s_sb = sb.tile([128, TCH], F32, tag="s_sb", bufs=3)
                    nc.scalar.activation(s_sb, s_ps, AF.Silu)
                    sr_sb = sb.tile([128, TCH], F32, tag="sr", bufs=3)
                    nc.vector.tensor_tensor(
                        sr_sb, s_sb, r_sb[:, tr * TCH : (tr + 1) * TCH], ALU.mult
                    )
                    nc.vector.tensor_tensor(m_all[:, i, :], sr_sb, agg_sb, ALU.mult)

                # output projection for this chunk: out[t, c] = sum_h m[h,t] Wout[h,c]
                c = max(i_ for i_ in range(NCC) if CC_START[i_] <= j)
                jr = j - CC_START[c]  # chunk index within the collective group
                for tb in range(TCH // 128):
                    tsl = slice(tb * 128, (tb + 1) * 128)
                    outp = [None] * (CO // 512)
                    for co in range(CO // 512):
                        outp[co] = ps.tile(
                            [128, 512], F32, tag=f"out_ps{co}", bufs=1, name=f"outp{co}"
                        )
                    # i outer / co inner: two matmuls share each stationary
                    # m tile, halving the fp32r weight-load overhead
                    for i in range(NHT):
                        for co in range(CO // 512):
                            nc.tensor.matmul(
                                outp[co],
                                m_all[:, i, tsl],
                                w_out_sb[:, i, co * 512 : (co + 1) * 512],
                                start=(i == 0),
                                stop=(i == NHT - 1),
                            )
                    # combine both co halves into one (128, 1024) tile so the
                    # partial write lands as full 4KB dram rows per partition
                    out_sb = sb.tile([128, CO], PDT, tag="out_sb", bufs=4)
                    for co in range(CO // 512):
                        nc.scalar.activation(
                            out_sb[:, co * 512 : (co + 1) * 512], outp[co], AF.Copy
                        )
                    # qACT HW queue: keep partial writes off the qSP queue
                    # so x-tile prefetches are never stuck behind them
                    nc.scalar.dma_start(
                        out=partials[c][
                            jr * TCH + tb * 128 : jr * TCH + (tb + 1) * 128, :
                        ],
                        in_=out_sb,
                    )

                # reduce-scatter a completed group of chunks across the TP group
                if j + 1 == CC_START[c] + CC_SCHED[c]:
                    nc.gpsimd.collective_compute(
                        "ReduceScatter",
                        ALU.add,
                        replica_groups=groups,
                        ins=[partials[c].opt()],
                        outs=[reds[c].opt()],
                    )
                    # gpsimd queue: a sync-queue DMA here would wait on the
                    # collective and head-of-line block later x-tile loads
                    r0 = CC_START[c] * TCH // TPW
                    nc.gpsimd.dma_start(
                        out=out_ext[r0 : r0 + CC_SCHED[c] * TCH // TPW, :],
                        in_=reds[c][:, :],
                    )

    nc.compile()
    return nc


_NC_CACHE = None


def _in_maps(x, W_sel, W_agg, W_out, sqk_q, sqk_k):
    x = np.asarray(x, dtype=np.float32)
    W_sel = np.asarray(W_sel, dtype=np.float32)
    W_agg = np.asarray(W_agg, dtype=np.float32)
    W_out = np.asarray(W_out, dtype=np.float32)
    sqk_q = np.asarray(sqk_q, dtype=np.float32)
    sqk_k = np.asarray(sqk_k, dtype=np.float32)

    # fold the per-channel scales into the projection weights
    W_sel_e = (W_sel * sqk_q[None, :]).astype(np.float16)
    W_agg_e = (W_agg * sqk_k[None, :]).astype(np.float16)
    W_out_r = W_out.astype(np.float16)
    x_r = x.astype(np.float16)

    r_bc = np.broadcast_to(
        (1.0 / np.arange(1, T + 1, dtype=np.float32))[None, :], (128, T)
    ).copy()

    def fold_k(a, ncol):
        # (C, ncol) -> (128, KC, ncol) with C = KC*128 split across partitions
        return np.ascontiguousarray(a.reshape(KC, 128, ncol).transpose(1, 0, 2))

    in_maps = []
    xt_cache = {}
    for k in range(N_CORES):
        g, q = k // TPW, k % TPW
        if g not in xt_cache:
            xg = np.concatenate([x_r[2 * g].T, x_r[2 * g + 1].T], axis=1)  # (C, TOK)
            # (NCH, 128, KC*TCH): per chunk, per partition, contiguous 16KB
            xt_cache[g] = np.ascontiguousarray(
                xg.reshape(KC, 128, NCH, TCH)
                .transpose(2, 1, 0, 3)
                .reshape(NCH, 128, KC * TCH)
            )
        in_maps.append(
            {
                "xt": xt_cache[g],
                "w_sel": fold_k(W_sel_e[:, q * HSH : (q + 1) * HSH], HSH),
                "w_c0": fold_k(W_agg_e[:, q * HSH : (q + 1) * HSH], HSH),
                "w_c1": fold_k(W_agg_e[:, H + q * HSH : H + (q + 1) * HSH], HSH),
                "w_out": np.ascontiguousarray(
                    W_out_r[q * HSH : (q + 1) * HSH, :]
                    .reshape(NHT, 128, CO)
                    .transpose(1, 0, 2)
                ),
                "r_bc": r_bc,
            }
        )
    return in_maps


def _assemble(res):
    # call c covers local tokens [CC_START[c]*TCH, ...); within the call the
    # ReduceScatter hands rank q the q-th quarter of those rows
    out = np.empty((B, T, C), dtype=np.float32)
    for g in range(NDP):
        for c in range(NCC):
            rows = CC_SCHED[c] * TCH // TPW
            r0 = CC_START[c] * TCH // TPW
            for q in range(TPW):
                piece = np.asarray(
                    res.results[TPW * g + q]["out"][r0 : r0 + rows], dtype=np.float32
                )
                lstart = CC_START[c] * TCH + q * rows
                b = 2 * g + lstart // T
                t0 = lstart % T
                out[b, t0 : t0 + rows] = piece
    return out


def kernel(x, W_sel, W_agg, W_out, sqk_q, sqk_k):
    global _NC_CACHE
    in_maps = _in_maps(x, W_sel, W_agg, W_out, sqk_q, sqk_k)
    if _NC_CACHE is None:
        _NC_CACHE = _build()
    res = run_bass_kernel_spmd(_NC_CACHE, in_maps, core_ids=list(range(N_CORES)))
    return _assemble(res)


def run_traced(x, W_sel, W_agg, W_out, sqk_q, sqk_k):
    """Timed run with NTFF trace; returns BassKernelResults (for test.py)."""
    global _NC_CACHE
    in_maps = _in_maps(x, W_sel, W_agg, W_out, sqk_q, sqk_k)
    if _NC_CACHE is None:
        _NC_CACHE = _build()
    return run_bass_kernel_spmd(
        _NC_CACHE,
        in_maps,
        core_ids=list(range(N_CORES)),
        trace=True,
        trace_cores=list(range(N_CORES)),
    )
